# revision 19
# baseline (speedup 1.0000x reference)
"""Trainium2 Bass kernel for nn_Dist_Conv2D (dist conv with conn-gather + inf-norm).

out[b,o,h,w] = max_j |weights[o,j] - x[b, c_j(o), clamp(h+dh_j(o)), clamp(w+dw_j(o))]| + bias[o]

Strategy (per core; data-parallel over batch, 8 cores x 4 batches):
  - Host precomputes, per batch, 96 row-shifted + W-padded + fully edge-clamped
    planes (c, dh) of x, stored as fp8 e4m3 hi/lo pairs (x = hi + lo), plus a
    constant ones plane. Layout: xp[b, p, sub, 1 + h*66 + k], 66-wide padded
    rows, 1-element margins so dw-shifted flat windows stay in bounds.
  - PE: per output tile (7 rows x 66 padded cols = 462 <= 512 PSUM bank), per
    tap j: 3 accumulating fp8 DoubleRow matmuls (dw = -1/0/+1 via flat window
    offsets; one-hot lhsT selects (c, dh); hi+lo contract via the two
    DoubleRow sub-rows; ones-row adds -w as a hi/lo pair). 0.5 cyc/col.
  - PSUM split: taps 0,1 -> psA pool (2 banks, bufs=2); tap 2 -> psB pool
    (1 bank, bufs=4). Tap-2 matmuls go last, so drains of taps 0,1 overlap
    the tail of the tile's own matmuls and psA frees early -- this keeps PE
    (the bottleneck engine) stall-free with only 8 PSUM banks.
  - Drains produce two bf16 streams (host combines max(S1,S2)+bias):
      A: Act Abs(psA strip) -> A01; Act Abs(psB) -> S2 rows directly;
         DVE tt max(A01[0],A01[1]) -> S1 rows.
      R: DVE reduce(max,|.|) over taps01 -> S1 rows; DVE stt max(-P2,P2) ->
         S2 rows.
  - ONE output DMA per stream per batch (HWDGE fixed cost is 625ns/DMA on a
    single-slot device, so DMA instruction count matters as much as bytes);
    y1 issued from Act, y2 from Pool, XP prefetched a batch ahead on SP.
"""

import sys

if "/opt/trn_rl_repo" not in sys.path:
    sys.path.insert(0, "/opt/trn_rl_repo")

import numpy as np
import ml_dtypes

FP8 = ml_dtypes.float8_e4m3
BF16 = ml_dtypes.bfloat16

import concourse.bass as bass
import concourse.mybir as mybir
from concourse import bacc, bass_utils
from concourse.tile import TileContext

B, C, H, W = 32, 32, 64, 64
OUT_C, CONN_NUM = 128, 3
NCORES = 8
BPC = B // NCORES
WP = W + 2                    # padded row width
RPT = 7                       # rows per full tile (7*66 = 462 <= 512 PSUM bank)
NFULL = 9                     # 9 full tiles cover 63 rows; 1 extra row
PLANE = H * WP                # 4224
PLANEB = PLANE + 2            # with 1-elem margins
NP_ = 97                      # 96 (c,dh) planes + ones row

# per-batch tile flavors (10 tiles): 'A' = Act-heavy, 'R' = DVE-reduce.
FLAVORS = ("A", "R", "A", "R", "A", "R", "A", "R", "A", "A")


def _tiles():
    ts = [(t * RPT, RPT) for t in range(NFULL)]
    ts.append((NFULL * RPT, 1))
    return ts


def _build_program():
    nc = bacc.Bacc("TRN2", target_bir_lowering=False, debug=False)
    f32 = mybir.dt.float32
    bf16 = mybir.dt.bfloat16
    fp8 = mybir.dt.float8e4
    Abs = mybir.ActivationFunctionType.Abs
    amax = mybir.AluOpType.max
    amult = mybir.AluOpType.mult
    DR = mybir.MatmulPerfMode.DoubleRow
    X = mybir.AxisListType.X

    xp = nc.dram_tensor("xp", [BPC, NP_, 2, PLANEB], fp8, kind="ExternalInput")
    lh = nc.dram_tensor("lh", [NP_, 3, 3, 2, 128], fp8, kind="ExternalInput")
    y1 = nc.dram_tensor("y1", [BPC, 128, H, W], bf16, kind="ExternalOutput")
    y2 = nc.dram_tensor("y2", [BPC, 128, H, W], bf16, kind="ExternalOutput")

    with TileContext(nc) as tc:
        with (
            tc.tile_pool(name="const", bufs=1) as cpool,
            tc.tile_pool(name="data", bufs=2) as dpool,
            tc.tile_pool(name="xpp", bufs=3) as xpool,
            tc.tile_pool(name="work", bufs=3) as wpool,
            tc.tile_pool(name="psA", bufs=3, space="PSUM") as papool,
            tc.tile_pool(name="psB", bufs=2, space="PSUM") as pbpool,
        ):
            LH = cpool.tile([NP_, 3, 3, 2, 128], fp8)
            nc.sync.dma_start(LH[:], lh[:])

            xps = {}

            def load_xp(b, split=False):
                xps[b] = xpool.tile([NP_, 2, PLANEB], fp8, name="XP", tag="XP")
                if split:
                    # halves: the first tiles only need low plane columns, so
                    # matmuls can start after half the transfer.
                    mid = PLANEB // 2
                    nc.sync.dma_start(xps[b][:, :, 0:mid], xp[b, :, :, 0:mid])
                    nc.sync.dma_start(xps[b][:, :, mid:], xp[b, :, :, mid:])
                else:
                    nc.sync.dma_start(xps[b][:], xp[b])

            load_xp(0, split=True)
            for b in range(BPC):
                # prefetch next batch's planes before this batch's stores are
                # queued anywhere, so the input DMA overlaps compute.
                if b + 1 < BPC:
                    load_xp(b + 1)
                XP = xps.pop(b)
                S1 = dpool.tile([128, H, W], bf16, name="S1", tag="S1")
                S2 = dpool.tile([128, H, W], bf16, name="S2", tag="S2")

                for t, (h0, rows) in enumerate(_tiles()):
                    L = rows * WP
                    PA = papool.tile([128, 2, 512], f32, name="PA", tag="PA")
                    PB = pbpool.tile([128, 512], f32, name="PB", tag="PB")
                    for j in range(3):
                        out_ap = PA[:, j, 0:L] if j < 2 else PB[:, 0:L]
                        for i, dwi in enumerate((0, 1, 2)):
                            off = 1 + h0 * WP + (dwi - 1)
                            nc.tensor.matmul(
                                out_ap,
                                LH[:, j, dwi],
                                XP[:, :, off : off + L],
                                start=(i == 0),
                                stop=(i == 2),
                                perf_mode=DR,
                            )

                    d1 = S1[:, h0 : h0 + rows, :]
                    d2 = S2[:, h0 : h0 + rows, :]
                    # [128, 2, L] -> [128, 2, rows, 64] strip view of taps 0,1
                    stripA = PA[:, :, 0:L].rearrange(
                        "p a (r c) -> p a r c", r=rows
                    )[:, :, :, 1 : 1 + W]
                    # [128, L] -> [128, rows, 64] strip view of tap 2
                    stripB = PB[:, 0:L].rearrange("p (r c) -> p r c", r=rows)[
                        :, :, 1 : 1 + W
                    ]
                    # psB (tap 2, 1 bank, bufs=2) must free fast: its drain is
                    # issued before the psA drain in both flavors.
                    if FLAVORS[t] == "A":
                        A01 = wpool.tile(
                            [128, 2, rows, W], bf16, name="A01", tag="A01"
                        )
                        nc.scalar.activation(d2, stripB, Abs)
                        nc.scalar.activation(A01[:], stripA, Abs)
                        nc.vector.tensor_tensor(d1, A01[:, 0], A01[:, 1], amax)
                    else:
                        # [128, 2, rows, 64] -> [128, rows, 64, 2] taps inner
                        rstrip = PA[:, :, 0:L].rearrange(
                            "p a (r c) -> p r c a", r=rows
                        )[:, :, 1 : 1 + W, :]
                        # |P2| via absmax-reduce over a singleton axis (stt
                        # can't read two PSUM operands)
                        rstripB = PB[:, 0:L].rearrange(
                            "p (r c u) -> p r c u", r=rows, u=1
                        )[:, :, 1 : 1 + W, :]
                        nc.vector.tensor_reduce(
                            d2, rstripB, X, amax, apply_absolute_value=True
                        )
                        nc.vector.tensor_reduce(
                            d1, rstrip, X, amax, apply_absolute_value=True
                        )

                    # stores flow through the batch: a completed row-band is
                    # shipped every 3 tiles, so the epilogue only ships the
                    # final single row and the DMA engine load is spread out.
                    # All on SP: its queue is otherwise idle (XP pool has 3
                    # bufs so the prefetch never parks on SP.SEQ).
                    if t in (2, 5, 8):
                        r0, r1 = (t - 2) * RPT, (t + 1) * RPT
                        nc.sync.dma_start(y1[b, :, r0:r1, :], S1[:, r0:r1, :])
                        nc.sync.dma_start(y2[b, :, r0:r1, :], S2[:, r0:r1, :])

                fr = NFULL * RPT
                nc.sync.dma_start(y1[b, :, fr:, :], S1[:, fr:, :])
                nc.sync.dma_start(y2[b, :, fr:, :], S2[:, fr:, :])
    nc.finalize()
    return nc


def _host_planes(x):
    """x: [B, C, H, W] f32 -> xp [B, NP_, 2, PLANEB] fp8 (hi/lo planes)."""
    n = x.shape[0]
    xw = np.empty((n, C, H, WP), np.float32)
    xw[:, :, :, 1 : 1 + W] = x
    xw[:, :, :, 0] = x[:, :, :, 0]
    xw[:, :, :, WP - 1] = x[:, :, :, W - 1]
    idx = np.arange(H)
    planes = np.empty((n, 3, C, H, WP), np.float32)
    for k, dh in enumerate((-1, 0, 1)):
        planes[:, k] = xw[:, :, np.clip(idx + dh, 0, H - 1), :]
    planes = planes.reshape(n, 96, PLANE)
    hi = planes.astype(FP8)
    lo = (planes - hi.astype(np.float32)).astype(FP8)
    xp = np.zeros((n, NP_, 2, PLANEB), FP8)
    xp[:, 0:96, 0, 1 : 1 + PLANE] = hi
    xp[:, 0:96, 1, 1 : 1 + PLANE] = lo
    xp[:, 96, :, :] = FP8(1.0)
    return xp


def _host_lhs(weights, conn):
    w = np.asarray(weights, np.float32).reshape(OUT_C, CONN_NUM)
    whi = w.astype(FP8).astype(np.float32)
    wlo = (w - whi).astype(FP8).astype(np.float32)
    lh = np.zeros((NP_, 3, 3, 2, 128), np.float32)
    conn = np.asarray(conn).reshape(OUT_C, CONN_NUM)
    for o in range(OUT_C):
        for j in range(CONN_NUM):
            v = int(conn[o, j])
            c, rem = divmod(v, 9)
            kh, kw = divmod(rem, 3)
            dh, dw = kh - 1, kw - 1
            lh[32 * (dh + 1) + c, j, dw + 1, 0, o] = 1.0
            lh[32 * (dh + 1) + c, j, dw + 1, 1, o] = 1.0
            lh[96, j, 1, 0, o] = -whi[o, j]
            lh[96, j, 1, 1, o] = -wlo[o, j]
    return lh.astype(FP8)


_NC_CACHE = []


def kernel(x, weights, bias, conn, _trace=False):
    x = np.asarray(x, np.float32)
    lhs = _host_lhs(weights, conn)
    xp = _host_planes(x)
    if not _NC_CACHE:
        _NC_CACHE.append(_build_program())
    nc = _NC_CACHE[0]
    in_maps = [
        {
            "xp": np.ascontiguousarray(xp[i * BPC : (i + 1) * BPC]),
            "lh": lhs,
        }
        for i in range(NCORES)
    ]
    res = bass_utils.run_bass_kernel_spmd(
        nc, in_maps, core_ids=list(range(NCORES)), trace=_trace
    )
    s1 = np.concatenate(
        [res.results[i]["y1"].astype(np.float32) for i in range(NCORES)], axis=0
    )
    s2 = np.concatenate(
        [res.results[i]["y2"].astype(np.float32) for i in range(NCORES)], axis=0
    )
    out = np.maximum(s1, s2) + np.asarray(bias, np.float32).reshape(1, OUT_C, 1, 1)
    if _trace:
        return out, res
    return out


# revision 21
# speedup vs baseline: 1.0122x; 1.0122x over previous
"""Trainium2 Bass kernel for nn_Dist_Conv2D (dist conv with conn-gather + inf-norm).

out[b,o,h,w] = max_j |weights[o,j] - x[b, c_j(o), clamp(h+dh_j(o)), clamp(w+dw_j(o))]| + bias[o]

Strategy (per core; data-parallel over batch, 8 cores x 4 batches):
  - Host precomputes, per batch, 96 row-shifted + W-padded + fully edge-clamped
    planes (c, dh) of x, stored as fp8 e4m3 hi/lo pairs (x = hi + lo), plus a
    constant ones plane. Layout: xp[b, p, sub, 1 + h*66 + k], 66-wide padded
    rows, 1-element margins so dw-shifted flat windows stay in bounds.
  - PE: per output tile (7 rows x 66 padded cols = 462 <= 512 PSUM bank), per
    tap j: 3 accumulating fp8 DoubleRow matmuls (dw = -1/0/+1 via flat window
    offsets; one-hot lhsT selects (c, dh); hi+lo contract via the two
    DoubleRow sub-rows; ones-row adds -w as a hi/lo pair). 0.5 cyc/col.
  - PSUM split: taps 0,1 -> psA pool (2 banks, bufs=2); tap 2 -> psB pool
    (1 bank, bufs=4). Tap-2 matmuls go last, so drains of taps 0,1 overlap
    the tail of the tile's own matmuls and psA frees early -- this keeps PE
    (the bottleneck engine) stall-free with only 8 PSUM banks.
  - Drains produce two bf16 streams (host combines max(S1,S2)+bias):
      A: Act Abs(psA strip) -> A01; Act Abs(psB) -> S2 rows directly;
         DVE tt max(A01[0],A01[1]) -> S1 rows.
      R: DVE reduce(max,|.|) over taps01 -> S1 rows; DVE stt max(-P2,P2) ->
         S2 rows.
  - ONE output DMA per stream per batch (HWDGE fixed cost is 625ns/DMA on a
    single-slot device, so DMA instruction count matters as much as bytes);
    y1 issued from Act, y2 from Pool, XP prefetched a batch ahead on SP.
"""

import sys

if "/opt/trn_rl_repo" not in sys.path:
    sys.path.insert(0, "/opt/trn_rl_repo")

import numpy as np
import ml_dtypes

FP8 = ml_dtypes.float8_e4m3
BF16 = ml_dtypes.bfloat16

import concourse.bass as bass
import concourse.mybir as mybir
from concourse import bacc, bass_utils
from concourse.tile import TileContext

B, C, H, W = 32, 32, 64, 64
OUT_C, CONN_NUM = 128, 3
NCORES = 8
BPC = B // NCORES
WP = W + 2                    # padded row width
RPT = 7                       # rows per full tile (7*66 = 462 <= 512 PSUM bank)
NFULL = 9                     # 9 full tiles cover 63 rows; 1 extra row
PLANE = H * WP                # 4224
PLANEB = PLANE + 2            # with 1-elem margins
NP_ = 97                      # 96 (c,dh) planes + ones row

# per-batch tile flavors (10 tiles): 'A' = Act-heavy, 'R' = DVE-reduce.
FLAVORS = ("A", "R", "A", "R", "A", "R", "A", "R", "A", "A")


def _tiles():
    ts = [(t * RPT, RPT) for t in range(NFULL)]
    ts.append((NFULL * RPT, 1))
    return ts


def _build_program():
    nc = bacc.Bacc("TRN2", target_bir_lowering=False, debug=False)
    f32 = mybir.dt.float32
    bf16 = mybir.dt.bfloat16
    fp8 = mybir.dt.float8e4
    Abs = mybir.ActivationFunctionType.Abs
    amax = mybir.AluOpType.max
    amult = mybir.AluOpType.mult
    DR = mybir.MatmulPerfMode.DoubleRow
    X = mybir.AxisListType.X

    xp = nc.dram_tensor("xp", [BPC, NP_, 2, PLANEB], fp8, kind="ExternalInput")
    lh = nc.dram_tensor("lh", [NP_, 3, 3, 2, 128], fp8, kind="ExternalInput")
    y1 = nc.dram_tensor("y1", [BPC, 128, H, W], bf16, kind="ExternalOutput")
    y2 = nc.dram_tensor("y2", [BPC, 128, H, W], bf16, kind="ExternalOutput")

    with TileContext(nc) as tc:
        with (
            tc.tile_pool(name="const", bufs=1) as cpool,
            tc.tile_pool(name="data", bufs=2) as dpool,
            tc.tile_pool(name="xpp", bufs=3) as xpool,
            tc.tile_pool(name="work", bufs=3) as wpool,
            tc.tile_pool(name="psA", bufs=3, space="PSUM") as papool,
            tc.tile_pool(name="psB", bufs=2, space="PSUM") as pbpool,
        ):
            LH = cpool.tile([NP_, 3, 3, 2, 128], fp8)
            nc.sync.dma_start(LH[:], lh[:])

            xps = {}

            def load_xp(b, split=False):
                xps[b] = xpool.tile([NP_, 2, PLANEB], fp8, name="XP", tag="XP")
                if split:
                    # halves: the first tiles only need low plane columns, so
                    # matmuls can start after half the transfer.
                    mid = PLANEB // 2
                    nc.sync.dma_start(xps[b][:, :, 0:mid], xp[b, :, :, 0:mid])
                    nc.sync.dma_start(xps[b][:, :, mid:], xp[b, :, :, mid:])
                else:
                    nc.sync.dma_start(xps[b][:], xp[b])

            load_xp(0, split=True)
            for b in range(BPC):
                # prefetch next batch's planes before this batch's stores are
                # queued anywhere, so the input DMA overlaps compute.
                if b + 1 < BPC:
                    load_xp(b + 1)
                XP = xps.pop(b)
                S1 = dpool.tile([128, H, W], bf16, name="S1", tag="S1")
                S2 = dpool.tile([128, H, W], bf16, name="S2", tag="S2")

                for t, (h0, rows) in enumerate(_tiles()):
                    L = rows * WP
                    PA = papool.tile([128, 2, 512], f32, name="PA", tag="PA")
                    PB = pbpool.tile([128, 512], f32, name="PB", tag="PB")
                    for j in range(3):
                        out_ap = PA[:, j, 0:L] if j < 2 else PB[:, 0:L]
                        for i, dwi in enumerate((0, 1, 2)):
                            off = 1 + h0 * WP + (dwi - 1)
                            nc.tensor.matmul(
                                out_ap,
                                LH[:, j, dwi],
                                XP[:, :, off : off + L],
                                start=(i == 0),
                                stop=(i == 2),
                                perf_mode=DR,
                            )

                    d1 = S1[:, h0 : h0 + rows, :]
                    d2 = S2[:, h0 : h0 + rows, :]
                    # [128, 2, L] -> [128, 2, rows, 64] strip view of taps 0,1
                    stripA = PA[:, :, 0:L].rearrange(
                        "p a (r c) -> p a r c", r=rows
                    )[:, :, :, 1 : 1 + W]
                    # [128, L] -> [128, rows, 64] strip view of tap 2
                    stripB = PB[:, 0:L].rearrange("p (r c) -> p r c", r=rows)[
                        :, :, 1 : 1 + W
                    ]
                    # psB (tap 2, 1 bank, bufs=2) must free fast: its drain is
                    # issued before the psA drain in both flavors.
                    if FLAVORS[t] == "A":
                        A01 = wpool.tile(
                            [128, 2, rows, W], bf16, name="A01", tag="A01"
                        )
                        nc.scalar.activation(A01[:], stripA, Abs)
                        nc.scalar.activation(d2, stripB, Abs)
                        nc.vector.tensor_tensor(d1, A01[:, 0], A01[:, 1], amax)
                    else:
                        # [128, 2, rows, 64] -> [128, rows, 64, 2] taps inner
                        rstrip = PA[:, :, 0:L].rearrange(
                            "p a (r c) -> p r c a", r=rows
                        )[:, :, 1 : 1 + W, :]
                        # |P2| via absmax-reduce over a singleton axis (stt
                        # can't read two PSUM operands)
                        rstripB = PB[:, 0:L].rearrange(
                            "p (r c u) -> p r c u", r=rows, u=1
                        )[:, :, 1 : 1 + W, :]
                        nc.vector.tensor_reduce(
                            d2, rstripB, X, amax, apply_absolute_value=True
                        )
                        nc.vector.tensor_reduce(
                            d1, rstrip, X, amax, apply_absolute_value=True
                        )

                    # stores flow through the batch: a completed row-band is
                    # shipped every 3 tiles, so the epilogue only ships the
                    # final single row and the DMA engine load is spread out.
                    # All on SP: its queue is otherwise idle (XP pool has 3
                    # bufs so the prefetch never parks on SP.SEQ).
                    if t in (2, 5):
                        r0, r1 = (t - 2) * RPT, (t + 1) * RPT
                        nc.sync.dma_start(y1[b, :, r0:r1, :], S1[:, r0:r1, :])
                        nc.sync.dma_start(y2[b, :, r0:r1, :], S2[:, r0:r1, :])
                    elif t == 8:
                        # only rows through t7 -- the final store then ships
                        # just 8 rows and overlaps the last tiles' drains
                        nc.sync.dma_start(
                            y1[b, :, 6 * RPT : 8 * RPT, :],
                            S1[:, 6 * RPT : 8 * RPT, :],
                        )
                        nc.sync.dma_start(
                            y2[b, :, 6 * RPT : 8 * RPT, :],
                            S2[:, 6 * RPT : 8 * RPT, :],
                        )

                fr = 8 * RPT
                nc.sync.dma_start(y1[b, :, fr:, :], S1[:, fr:, :])
                nc.sync.dma_start(y2[b, :, fr:, :], S2[:, fr:, :])
    nc.finalize()
    return nc


def _host_planes(x):
    """x: [B, C, H, W] f32 -> xp [B, NP_, 2, PLANEB] fp8 (hi/lo planes)."""
    n = x.shape[0]
    xw = np.empty((n, C, H, WP), np.float32)
    xw[:, :, :, 1 : 1 + W] = x
    xw[:, :, :, 0] = x[:, :, :, 0]
    xw[:, :, :, WP - 1] = x[:, :, :, W - 1]
    idx = np.arange(H)
    planes = np.empty((n, 3, C, H, WP), np.float32)
    for k, dh in enumerate((-1, 0, 1)):
        planes[:, k] = xw[:, :, np.clip(idx + dh, 0, H - 1), :]
    planes = planes.reshape(n, 96, PLANE)
    hi = planes.astype(FP8)
    lo = (planes - hi.astype(np.float32)).astype(FP8)
    xp = np.zeros((n, NP_, 2, PLANEB), FP8)
    xp[:, 0:96, 0, 1 : 1 + PLANE] = hi
    xp[:, 0:96, 1, 1 : 1 + PLANE] = lo
    xp[:, 96, :, :] = FP8(1.0)
    return xp


def _host_lhs(weights, conn):
    w = np.asarray(weights, np.float32).reshape(OUT_C, CONN_NUM)
    whi = w.astype(FP8).astype(np.float32)
    wlo = (w - whi).astype(FP8).astype(np.float32)
    lh = np.zeros((NP_, 3, 3, 2, 128), np.float32)
    conn = np.asarray(conn).reshape(OUT_C, CONN_NUM)
    for o in range(OUT_C):
        for j in range(CONN_NUM):
            v = int(conn[o, j])
            c, rem = divmod(v, 9)
            kh, kw = divmod(rem, 3)
            dh, dw = kh - 1, kw - 1
            lh[32 * (dh + 1) + c, j, dw + 1, 0, o] = 1.0
            lh[32 * (dh + 1) + c, j, dw + 1, 1, o] = 1.0
            lh[96, j, 1, 0, o] = -whi[o, j]
            lh[96, j, 1, 1, o] = -wlo[o, j]
    return lh.astype(FP8)


_NC_CACHE = []


def kernel(x, weights, bias, conn, _trace=False):
    x = np.asarray(x, np.float32)
    lhs = _host_lhs(weights, conn)
    xp = _host_planes(x)
    if not _NC_CACHE:
        _NC_CACHE.append(_build_program())
    nc = _NC_CACHE[0]
    in_maps = [
        {
            "xp": np.ascontiguousarray(xp[i * BPC : (i + 1) * BPC]),
            "lh": lhs,
        }
        for i in range(NCORES)
    ]
    res = bass_utils.run_bass_kernel_spmd(
        nc, in_maps, core_ids=list(range(NCORES)), trace=_trace
    )
    s1 = np.concatenate(
        [res.results[i]["y1"].astype(np.float32) for i in range(NCORES)], axis=0
    )
    s2 = np.concatenate(
        [res.results[i]["y2"].astype(np.float32) for i in range(NCORES)], axis=0
    )
    out = np.maximum(s1, s2) + np.asarray(bias, np.float32).reshape(1, OUT_C, 1, 1)
    if _trace:
        return out, res
    return out
